# revision 11
# baseline (speedup 1.0000x reference)
"""CLUB loss kernel for Trainium2, 8 NeuronCores (SPMD data-parallel).

Math: with flat_x (N,d), iv = exp(-p_logvar):
  positive_i = -0.5 * sum_d (x_i - mu_i)^2 * iv_i
  negative_i = -0.5 * sum_d iv_i * (ex2 - 2 mu_i ex + mu_i^2),  ex/ex2 = col-moments of flat_x
  loss = mean_i(positive_i - negative_i)
Decomposed into global sums (single pass over data):
  sx[d]  = sum_i x,   sxx[d] = sum_i x^2
  A[d]   = sum_i iv,  B2[d]  = sum_i iv*mu      (host folds the 2x)
  Ta     = sum iv*x^2,  Tb = sum iv*mu*x,  T = Ta - 2*Tb
  loss = -0.5/N * [ T - dot(sxx,A)/N + dot(sx,2*B2)/N ]

v4 implementation notes:
- mu/logvar are DMA'd with 4 consecutive rows packed per partition line
  (2KB contiguous packets, 4x fewer DMA descriptors). The TensorEngine
  transpose of each 128x128 block writes its PSUM columns with stride 4
  (offset k), which exactly undoes the packing interleave: the PSUM
  chunk ends up in natural row order, so ALL downstream ops use plain
  contiguous APs.
- ACT (fp16 outs, fp32 fused accumulators):
    iv  = exp(-lvT)   + accum A      (1024-wide: lvT spans 2 PSUM banks)
    xsq = Square(x)   + accum sxx
    x16 = Copy(x)     + accum sx
- DVE scalar_tensor_tensor (1x rate, fuses product + reduction):
    j  = iv * muT(PSUM)  + accum B2   (no separate mu drain pass)
    t1 = iv * xsq        + accum Ta
    t2 = j16 * x         + accum Tb
- Constraints found by bisection on this runtime: tensor_scalar+accum /
  gpsimd stt / tensor_tensor_reduce crash neuronxcc; stt operands allow
  at most 2 free dims; DVE 2x/4x perf modes do not apply to stt or
  reduce ops (so fp16 buys no rate, but halves SBUF traffic); gpsimd
  tensor_reduce cannot reduce the free axis.
- fp16 intermediates are safe: every sum accumulates in fp32 before
  rounding (measured pipeline rel err ~3e-3 vs 2e-2 tolerance).
- Each core emits a (128,6) stats block; final O(d) combine on host.
"""

import numpy as np

B, D, H, W = 16, 128, 64, 64
N = B * H * W            # 65536
NCORES = 8
BPC = B // NCORES        # 2 batches per core
HW = H * W               # 4096
ROWS = BPC * HW          # 8192 rows per core
CHUNK = 512              # i-rows per transpose/PSUM group
CPB = HW // CHUNK        # 8 chunks per batch
NCHUNK = BPC * CPB       # 16 chunks per core
HB = HW // 2             # 2048 rows per half-batch
NHB = 2 * BPC            # 4 half-batches per core
KPACK = 4                # mu/lv rows packed per partition DMA line

_CACHE = {}


def _build_nc():
    import concourse.bacc as bacc
    import concourse.mybir as mybir
    from concourse import masks
    from concourse.tile import TileContext

    f32 = mybir.dt.float32
    f16 = mybir.dt.float16
    ALU = mybir.AluOpType
    AF = mybir.ActivationFunctionType
    AX = mybir.AxisListType

    nc = bacc.Bacc(num_devices=NCORES)
    x_in = nc.dram_tensor("x", [BPC, D, HW], f32, kind="ExternalInput")
    mu_in = nc.dram_tensor("p_mu", [ROWS, D], f32, kind="ExternalInput")
    lv_in = nc.dram_tensor("p_logvar", [ROWS, D], f32, kind="ExternalInput")
    stats_out = nc.dram_tensor("stats", [128, 6], f32, kind="ExternalOutput")

    with TileContext(nc) as tc:
        with (
            tc.tile_pool(name="const", bufs=1) as constp,
            tc.tile_pool(name="slabs", bufs=2) as slabs,
            tc.tile_pool(name="work", bufs=2) as work,
            tc.tile_pool(name="stats", bufs=1) as stats,
            tc.tile_pool(name="psmu", bufs=3, space="PSUM") as psmu,
            tc.tile_pool(name="pslv", bufs=3, space="PSUM") as pslv,
        ):
            ident = constp.tile([128, 128], f32, name="ident")
            masks.make_identity(nc, ident[:])

            A_cols = stats.tile([128, NCHUNK], f32, name="A_cols")
            B2_cols = stats.tile([128, NCHUNK], f32, name="B2_cols")
            sx_cols = stats.tile([128, NHB], f32, name="sx_cols")
            sxx_cols = stats.tile([128, NHB], f32, name="sxx_cols")
            Ta_cols = stats.tile([128, NHB], f32, name="Ta_cols")
            Tb_cols = stats.tile([128, NHB], f32, name="Tb_cols")

            G = HB // CHUNK  # 4 groups of 512 rows per half-slab
            for b in range(BPC):
                x_slab = slabs.tile([128, HW], f32, tag="x_slab",
                                    name="x_slab")
                mu_slab = slabs.tile([128, HW], f32, tag="mu_slab",
                                     name="mu_slab")
                lv_slab = slabs.tile([128, HW], f32, tag="lv_slab",
                                     name="lv_slab")
                # mu/lv pack KPACK=4 consecutive rows per partition line
                # -> 2KB contiguous packets. Slab free layout per group
                # g: (k,d): row g*512 + p*4 + k at partition p, free
                # g*512 + k*128 + d.
                for h in range(2):
                    nc.sync.dma_start(out=x_slab[:, h * HB:(h + 1) * HB],
                                      in_=x_in[b, :, h * HB:(h + 1) * HB])
                    r0 = b * HW + h * HB
                    nc.sync.dma_start(
                        out=mu_slab[:, h * HB:(h + 1) * HB].rearrange(
                            "p (g k d) -> p g k d", g=G, k=KPACK),
                        in_=mu_in[r0:r0 + HB, :].rearrange(
                            "(g p k) d -> p g k d", g=G, p=128),
                    )
                    nc.sync.dma_start(
                        out=lv_slab[:, h * HB:(h + 1) * HB].rearrange(
                            "p (g k d) -> p g k d", g=G, k=KPACK),
                        in_=lv_in[r0:r0 + HB, :].rearrange(
                            "(g p k) d -> p g k d", g=G, p=128),
                    )

                for h in range(2):
                    hb = b * 2 + h
                    iv = work.tile([128, HB], f16, tag="iv", name="iv")
                    xsq = work.tile([128, HB], f16, tag="xsq", name="xsq")
                    x16 = work.tile([128, HB], f16, tag="x16", name="x16")
                    j16 = work.tile([128, HB], f16, tag="j16", name="j16")
                    scr1 = work.tile([128, HB], f16, tag="scr1", name="scr1")
                    scr2 = work.tile([128, HB], f16, tag="scr2", name="scr2")

                    xh = x_slab[:, h * HB:(h + 1) * HB]
                    # ACT: sx and xsq/sxx via fused accum
                    nc.scalar.activation(
                        x16[:], xh, AF.Copy,
                        accum_out=sx_cols[:, hb:hb + 1])
                    nc.scalar.activation(
                        xsq[:], xh, AF.Square,
                        accum_out=sxx_cols[:, hb:hb + 1])

                    # lvT spans 2 groups (2 PSUM banks) per tile so exp
                    # runs 1024-wide; muT is 1 bank, j runs 512-wide.
                    for g in range(G):
                        cc = h * G + g
                        ci = b * CPB + cc
                        muT = psmu.tile([128, CHUNK], f32, tag="muT",
                                        name="muT")
                        lvT = pslv.tile([128, CHUNK], f32, tag="lvT",
                                        name="lvT")
                        # strided PSUM views: transpose block k writes
                        # cols k, k+4, ... undoing the DMA packing ->
                        # natural row order in PSUM
                        muTv = muT[:].rearrange("p (c k) -> p k c",
                                                c=128, k=KPACK)
                        lvTv = lvT[:].rearrange("p (c k) -> p k c",
                                                c=128, k=KPACK)
                        for k in range(KPACK):
                            col = cc * CHUNK + k * 128
                            nc.tensor.transpose(
                                muTv[:, k, :],
                                mu_slab[:, col:col + 128], ident[:])
                            nc.tensor.transpose(
                                lvTv[:, k, :],
                                lv_slab[:, col:col + 128], ident[:])
                        sl = slice(g * CHUNK, (g + 1) * CHUNK)
                        # ACT: iv = exp(-lvT) fp16 + fused A accum
                        nc.scalar.activation(
                            iv[:, sl], lvT[:], AF.Exp, bias=0.0,
                            scale=-1.0,
                            accum_out=A_cols[:, ci:ci + 1])
                        # DVE: j = iv*muT (straight from PSUM) + B2
                        nc.vector.scalar_tensor_tensor(
                            out=j16[:, sl], in0=iv[:, sl], scalar=1.0,
                            in1=muT[:], op0=ALU.mult, op1=ALU.mult,
                            accum_out=B2_cols[:, ci:ci + 1])

                    # DVE: t1 = iv*xsq + Ta   (all fp16, contiguous)
                    nc.vector.scalar_tensor_tensor(
                        out=scr1[:], in0=iv[:], scalar=1.0, in1=xsq[:],
                        op0=ALU.mult, op1=ALU.mult,
                        accum_out=Ta_cols[:, hb:hb + 1])
                    # DVE: t2 = j*x16 + Tb
                    nc.vector.scalar_tensor_tensor(
                        out=scr2[:], in0=j16[:], scalar=1.0, in1=x16[:],
                        op0=ALU.mult, op1=ALU.mult,
                        accum_out=Tb_cols[:, hb:hb + 1])

            # ---- fold partial columns to (128,1) each ----
            g6 = stats.tile([128, 6], f32, name="g6")
            nc.vector.tensor_reduce(g6[:, 0:1], sx_cols[:], axis=AX.X,
                                    op=ALU.add)
            nc.vector.tensor_reduce(g6[:, 1:2], sxx_cols[:], axis=AX.X,
                                    op=ALU.add)
            nc.vector.tensor_reduce(g6[:, 2:3], A_cols[:], axis=AX.X,
                                    op=ALU.add)
            nc.vector.tensor_reduce(g6[:, 3:4], B2_cols[:], axis=AX.X,
                                    op=ALU.add)
            nc.vector.tensor_reduce(g6[:, 4:5], Ta_cols[:], axis=AX.X,
                                    op=ALU.add)
            nc.vector.tensor_reduce(g6[:, 5:6], Tb_cols[:], axis=AX.X,
                                    op=ALU.add)
            nc.sync.dma_start(out=stats_out[:], in_=g6[:])

    return nc


def get_nc(**_ignored):
    key = "nc"
    if key not in _CACHE:
        nc = _build_nc()
        if not nc.is_finalized():
            nc.finalize()
        _CACHE[key] = nc
    return _CACHE[key]


def make_in_maps(x, p_mu, p_logvar):
    x = np.ascontiguousarray(np.asarray(x, dtype=np.float32))
    p_mu = np.ascontiguousarray(np.asarray(p_mu, dtype=np.float32))
    p_logvar = np.ascontiguousarray(np.asarray(p_logvar, dtype=np.float32))
    in_maps = []
    for c in range(NCORES):
        in_maps.append({
            "x": np.ascontiguousarray(
                x[c * BPC:(c + 1) * BPC].reshape(BPC, D, HW)),
            "p_mu": np.ascontiguousarray(p_mu[c * ROWS:(c + 1) * ROWS]),
            "p_logvar": np.ascontiguousarray(
                p_logvar[c * ROWS:(c + 1) * ROWS]),
        })
    return in_maps


MODE = "host"


def kernel(x, p_mu, p_logvar):
    from concourse.bass_utils import run_bass_kernel_spmd

    in_maps = make_in_maps(x, p_mu, p_logvar)
    # device computes per-core stats partials (sx, sxx, A, B2, Ta, Tb per
    # channel); the final O(d) reduction of the 8 blocks happens here.
    nc = get_nc()
    res = run_bass_kernel_spmd(nc, in_maps, list(range(NCORES)))
    s = np.zeros((128, 6), dtype=np.float64)
    for c in range(NCORES):
        s += np.asarray(res.results[c]["stats"], dtype=np.float64)
    sx, sxx, A, B2p, Ta, Tb = (s[:, k] for k in range(6))
    # B2p/Tb carry iv*mu (not 2*iv*mu): fold the 2x here
    T = Ta.sum() - 2.0 * Tb.sum()
    loss = -0.5 / N * (T - sxx.dot(A) / N + sx.dot(2.0 * B2p) / N)
    return np.asarray(loss, dtype=np.float32).reshape(())


# revision 13
# speedup vs baseline: 1.0298x; 1.0298x over previous
"""CLUB loss kernel for Trainium2, 8 NeuronCores (SPMD data-parallel).

Math: with flat_x (N,d), iv = exp(-p_logvar):
  positive_i = -0.5 * sum_d (x_i - mu_i)^2 * iv_i
  negative_i = -0.5 * sum_d iv_i * (ex2 - 2 mu_i ex + mu_i^2),  ex/ex2 = col-moments of flat_x
  loss = mean_i(positive_i - negative_i)
Decomposed into global sums (single pass over data):
  sx[d]  = sum_i x,   sxx[d] = sum_i x^2
  A[d]   = sum_i iv,  B2[d]  = sum_i iv*mu      (host folds the 2x)
  Ta     = sum iv*x^2,  Tb = sum iv*mu*x,  T = Ta - 2*Tb
  loss = -0.5/N * [ T - dot(sxx,A)/N + dot(sx,2*B2)/N ]

v4 implementation notes:
- mu/logvar are DMA'd with 4 consecutive rows packed per partition line
  (2KB contiguous packets, 4x fewer DMA descriptors). The TensorEngine
  transpose of each 128x128 block writes its PSUM columns with stride 4
  (offset k), which exactly undoes the packing interleave: the PSUM
  chunk ends up in natural row order, so ALL downstream ops use plain
  contiguous APs.
- ACT (fp16 outs, fp32 fused accumulators):
    iv  = exp(-lvT)   + accum A      (1024-wide: lvT spans 2 PSUM banks)
    xsq = Square(x)   + accum sxx
    x16 = Copy(x)     + accum sx
- DVE scalar_tensor_tensor (1x rate, fuses product + reduction):
    j  = iv * muT(PSUM)  + accum B2   (no separate mu drain pass)
    t1 = iv * xsq        + accum Ta
    t2 = j16 * x         + accum Tb
- Constraints found by bisection on this runtime: tensor_scalar+accum /
  gpsimd stt / tensor_tensor_reduce crash neuronxcc; stt operands allow
  at most 2 free dims; DVE 2x/4x perf modes do not apply to stt or
  reduce ops (so fp16 buys no rate, but halves SBUF traffic); gpsimd
  tensor_reduce cannot reduce the free axis.
- fp16 intermediates are safe: every sum accumulates in fp32 before
  rounding (measured pipeline rel err ~3e-3 vs 2e-2 tolerance).
- Each core emits a (128,6) stats block; final O(d) combine on host.
"""

import numpy as np

B, D, H, W = 16, 128, 64, 64
N = B * H * W            # 65536
NCORES = 8
BPC = B // NCORES        # 2 batches per core
HW = H * W               # 4096
ROWS = BPC * HW          # 8192 rows per core
CHUNK = 512              # i-rows per transpose/PSUM group
CPB = HW // CHUNK        # 8 chunks per batch
NCHUNK = BPC * CPB       # 16 chunks per core
HB = HW // 2             # 2048 rows per half-batch
NHB = 2 * BPC            # 4 half-batches per core
KPACK = 4                # mu/lv rows packed per partition DMA line

_CACHE = {}


def _build_nc():
    import concourse.bacc as bacc
    import concourse.mybir as mybir
    from concourse import masks
    from concourse.tile import TileContext

    f32 = mybir.dt.float32
    f16 = mybir.dt.float16
    ALU = mybir.AluOpType
    AF = mybir.ActivationFunctionType
    AX = mybir.AxisListType

    nc = bacc.Bacc(num_devices=NCORES)
    x_in = nc.dram_tensor("x", [BPC, D, HW], f32, kind="ExternalInput")
    mu_in = nc.dram_tensor("p_mu", [ROWS, D], f32, kind="ExternalInput")
    lv_in = nc.dram_tensor("p_logvar", [ROWS, D], f32, kind="ExternalInput")
    stats_out = nc.dram_tensor("stats", [128, 6], f32, kind="ExternalOutput")

    with TileContext(nc) as tc:
        with (
            tc.tile_pool(name="const", bufs=1) as constp,
            tc.tile_pool(name="slabs", bufs=2) as slabs,
            tc.tile_pool(name="work", bufs=2) as work,
            tc.tile_pool(name="stats", bufs=1) as stats,
            tc.tile_pool(name="psmu", bufs=3, space="PSUM") as psmu,
            tc.tile_pool(name="pslv", bufs=3, space="PSUM") as pslv,
        ):
            ident = constp.tile([128, 128], f32, name="ident")
            masks.make_identity(nc, ident[:])

            A_cols = stats.tile([128, NCHUNK], f32, name="A_cols")
            B2_cols = stats.tile([128, NCHUNK], f32, name="B2_cols")
            sx_cols = stats.tile([128, NHB], f32, name="sx_cols")
            sxx_cols = stats.tile([128, NHB], f32, name="sxx_cols")
            Ta_cols = stats.tile([128, NHB], f32, name="Ta_cols")
            Tb_cols = stats.tile([128, NHB], f32, name="Tb_cols")

            G = HB // CHUNK  # 4 groups of 512 rows per half-slab
            for b in range(BPC):
                x_slab = slabs.tile([128, HW], f32, tag="x_slab",
                                    name="x_slab")
                mu_slab = slabs.tile([128, HW], f32, tag="mu_slab",
                                     name="mu_slab")
                lv_slab = slabs.tile([128, HW], f32, tag="lv_slab",
                                     name="lv_slab")
                # mu/lv pack KPACK=4 consecutive rows per partition line
                # -> 2KB contiguous packets. Slab free layout per group
                # g: (k,d): row g*512 + p*4 + k at partition p, free
                # g*512 + k*128 + d.
                for h in range(2):
                    # mu/lv first: they gate the transpose->exp->j chain;
                    # x is consumed later in the per-half-batch pipeline
                    r0 = b * HW + h * HB
                    nc.sync.dma_start(
                        out=mu_slab[:, h * HB:(h + 1) * HB].rearrange(
                            "p (g k d) -> p g k d", g=G, k=KPACK),
                        in_=mu_in[r0:r0 + HB, :].rearrange(
                            "(g p k) d -> p g k d", g=G, p=128),
                    )
                    nc.sync.dma_start(
                        out=lv_slab[:, h * HB:(h + 1) * HB].rearrange(
                            "p (g k d) -> p g k d", g=G, k=KPACK),
                        in_=lv_in[r0:r0 + HB, :].rearrange(
                            "(g p k) d -> p g k d", g=G, p=128),
                    )
                    nc.sync.dma_start(out=x_slab[:, h * HB:(h + 1) * HB],
                                      in_=x_in[b, :, h * HB:(h + 1) * HB])

                for h in range(2):
                    hb = b * 2 + h
                    iv = work.tile([128, HB], f16, tag="iv", name="iv")
                    xsq = work.tile([128, HB], f16, tag="xsq", name="xsq")
                    x16 = work.tile([128, HB], f16, tag="x16", name="x16")
                    j16 = work.tile([128, HB], f16, tag="j16", name="j16")
                    scr1 = work.tile([128, HB], f16, tag="scr1", name="scr1")
                    scr2 = work.tile([128, HB], f16, tag="scr2", name="scr2")

                    xh = x_slab[:, h * HB:(h + 1) * HB]

                    def do_chunk(g):
                        cc = h * G + g
                        ci = b * CPB + cc
                        muT = psmu.tile([128, CHUNK], f32, tag="muT",
                                        name="muT")
                        lvT = pslv.tile([128, CHUNK], f32, tag="lvT",
                                        name="lvT")
                        # strided PSUM views: transpose block k writes
                        # cols k, k+4, ... undoing the DMA packing ->
                        # natural row order in PSUM
                        muTv = muT[:].rearrange("p (c k) -> p k c",
                                                c=128, k=KPACK)
                        lvTv = lvT[:].rearrange("p (c k) -> p k c",
                                                c=128, k=KPACK)
                        for k in range(KPACK):
                            col = cc * CHUNK + k * 128
                            nc.tensor.transpose(
                                muTv[:, k, :],
                                mu_slab[:, col:col + 128], ident[:])
                            nc.tensor.transpose(
                                lvTv[:, k, :],
                                lv_slab[:, col:col + 128], ident[:])
                        sl = slice(g * CHUNK, (g + 1) * CHUNK)
                        # ACT: iv = exp(-lvT) fp16 + fused A accum
                        nc.scalar.activation(
                            iv[:, sl], lvT[:], AF.Exp, bias=0.0,
                            scale=-1.0,
                            accum_out=A_cols[:, ci:ci + 1])
                        # DVE: j = iv*muT (straight from PSUM) + B2
                        nc.vector.scalar_tensor_tensor(
                            out=j16[:, sl], in0=iv[:, sl], scalar=1.0,
                            in1=muT[:], op0=ALU.mult, op1=ALU.mult,
                            accum_out=B2_cols[:, ci:ci + 1])

                    # ACT queue order: exp g0, exp g1, Square, exp g2,
                    # exp g3, Copy -- the exps gate the DVE j chain, the
                    # Square gates t1; sx's Copy is pure bookkeeping and
                    # goes last (t2 reads fp32 x directly).
                    do_chunk(0)
                    do_chunk(1)
                    nc.scalar.activation(
                        xsq[:], xh, AF.Square,
                        accum_out=sxx_cols[:, hb:hb + 1])
                    do_chunk(2)
                    do_chunk(3)
                    # DVE: t1 = iv*xsq + Ta   (all fp16, contiguous)
                    nc.vector.scalar_tensor_tensor(
                        out=scr1[:], in0=iv[:], scalar=1.0, in1=xsq[:],
                        op0=ALU.mult, op1=ALU.mult,
                        accum_out=Ta_cols[:, hb:hb + 1])
                    # DVE: t2 = j * x (fp32 in1) + Tb
                    nc.vector.scalar_tensor_tensor(
                        out=scr2[:], in0=j16[:], scalar=1.0, in1=xh,
                        op0=ALU.mult, op1=ALU.mult,
                        accum_out=Tb_cols[:, hb:hb + 1])
                    # ACT: sx via fused accum on a fp16 cast
                    nc.scalar.activation(
                        x16[:], xh, AF.Copy,
                        accum_out=sx_cols[:, hb:hb + 1])

            # ---- fold partial columns to (128,1) each ----
            g6 = stats.tile([128, 6], f32, name="g6")
            nc.vector.tensor_reduce(g6[:, 0:1], sx_cols[:], axis=AX.X,
                                    op=ALU.add)
            nc.vector.tensor_reduce(g6[:, 1:2], sxx_cols[:], axis=AX.X,
                                    op=ALU.add)
            nc.vector.tensor_reduce(g6[:, 2:3], A_cols[:], axis=AX.X,
                                    op=ALU.add)
            nc.vector.tensor_reduce(g6[:, 3:4], B2_cols[:], axis=AX.X,
                                    op=ALU.add)
            nc.vector.tensor_reduce(g6[:, 4:5], Ta_cols[:], axis=AX.X,
                                    op=ALU.add)
            nc.vector.tensor_reduce(g6[:, 5:6], Tb_cols[:], axis=AX.X,
                                    op=ALU.add)
            nc.sync.dma_start(out=stats_out[:], in_=g6[:])

    return nc


def get_nc(**_ignored):
    key = "nc"
    if key not in _CACHE:
        nc = _build_nc()
        if not nc.is_finalized():
            nc.finalize()
        _CACHE[key] = nc
    return _CACHE[key]


def make_in_maps(x, p_mu, p_logvar):
    x = np.ascontiguousarray(np.asarray(x, dtype=np.float32))
    p_mu = np.ascontiguousarray(np.asarray(p_mu, dtype=np.float32))
    p_logvar = np.ascontiguousarray(np.asarray(p_logvar, dtype=np.float32))
    in_maps = []
    for c in range(NCORES):
        in_maps.append({
            "x": np.ascontiguousarray(
                x[c * BPC:(c + 1) * BPC].reshape(BPC, D, HW)),
            "p_mu": np.ascontiguousarray(p_mu[c * ROWS:(c + 1) * ROWS]),
            "p_logvar": np.ascontiguousarray(
                p_logvar[c * ROWS:(c + 1) * ROWS]),
        })
    return in_maps


MODE = "host"


def kernel(x, p_mu, p_logvar):
    from concourse.bass_utils import run_bass_kernel_spmd

    in_maps = make_in_maps(x, p_mu, p_logvar)
    # device computes per-core stats partials (sx, sxx, A, B2, Ta, Tb per
    # channel); the final O(d) reduction of the 8 blocks happens here.
    nc = get_nc()
    res = run_bass_kernel_spmd(nc, in_maps, list(range(NCORES)))
    s = np.zeros((128, 6), dtype=np.float64)
    for c in range(NCORES):
        s += np.asarray(res.results[c]["stats"], dtype=np.float64)
    sx, sxx, A, B2p, Ta, Tb = (s[:, k] for k in range(6))
    # B2p/Tb carry iv*mu (not 2*iv*mu): fold the 2x here
    T = Ta.sum() - 2.0 * Tb.sum()
    loss = -0.5 / N * (T - sxx.dot(A) / N + sx.dot(2.0 * B2p) / N)
    return np.asarray(loss, dtype=np.float32).reshape(())
